# revision 6
# baseline (speedup 1.0000x reference)
"""Trainium2 Bass kernel for nn_AttentionMechanism (location-sensitive additive
attention, B=32 T=1500 E=D=A=512, conv C=10 K=201).

Strategy (8 NeuronCores, data-parallel over batch, 4 batches/core):
  per batch b:
    encT = enc[b].T             (PE transposes of bf16 tiles)
    pre.T[a, t] = sum_e W_enc[e,a] enc[t,e] + sum_k G[k,a] aw_pad[t+k]
                  (+ per-a bias = dec[b] @ W_dec + b_enc, folded into tanh)
    tanhT = tanh(pre.T + bias)  (ScalarE, PSUM -> SBUF bf16)
    energy[t] = sum_a V[a] tanhT[a, t]   (PE, m=1 rows at partition 32b)
  softmax over T for all 4 batches at once (partitions {0,32,64,96})
  context[b] = sum_t aw[t] enc[b,t,:]    (PE, aw.T tiles as lhsT)

G = einsum('ck,ca->ka', conv_w[:,0,:], W_conv) is folded on the host
(weights only).  The conv input is the Hankel matrix S[k,t] = aw_pad[t+k],
built by an overlapping-window DMA directly from DRAM.

kernel(**inputs) takes the FULL unsharded inputs (names as in
reference.setup_inputs) and returns (context [B,1,E], aw [B,T,1]).
"""

import numpy as np
import ml_dtypes

import concourse.bacc as bacc
import concourse.mybir as mybir
import concourse.tile as tile
from concourse.bass_types import AP
from concourse.bass_utils import run_bass_kernel_spmd
from concourse.masks import make_identity

F32 = mybir.dt.float32
BF16 = mybir.dt.bfloat16

B, T, E, D, A, C, KW = 32, 1500, 512, 512, 512, 10, 201
NCORES = 8
NB = B // NCORES              # batches per core
TPAD = 1704                   # aw_pad length (>= T + KW - 1 = 1700)
NT = 12                       # 128-row tiles of T (last is 92)
TSZ = [128] * 11 + [92]
NCH = 3                       # chunks of T for matmul free dim
CH = 500
KA = A // 128                 # 4
KE = E // 128
KD = D // 128
KW1, KW2 = 128, KW - 128      # Hankel k-tiles: 128 + 73

# chunk -> t-tiles whose transposes must exist before the chunk's matmuls
CHUNK_TILES = [[0, 1, 2, 3], [4, 5, 6, 7], [8, 9, 10, 11]]

TRACE = False
LAST_EXEC_NS = None
LAST_RESULTS = None


def _build():
    nc = bacc.Bacc(trn_type="TRN2", debug=False)

    enc = nc.dram_tensor("enc", [NB, T, E], F32, kind="ExternalInput").ap()
    dec = nc.dram_tensor("dec", [NB, D], F32, kind="ExternalInput").ap()
    awp = nc.dram_tensor("awp", [NB, TPAD], BF16, kind="ExternalInput").ap()
    msk = nc.dram_tensor("msk", [NB, T], F32, kind="ExternalInput").ap()
    wenc = nc.dram_tensor("wenc", [E, A], BF16, kind="ExternalInput").ap()
    wdec = nc.dram_tensor("wdec", [D, A], BF16, kind="ExternalInput").ap()
    gmat = nc.dram_tensor("gmat", [KW, A], BF16, kind="ExternalInput").ap()
    vt = nc.dram_tensor("vt", [128, KA], BF16, kind="ExternalInput").ap()
    benc = nc.dram_tensor("benc", [1, A], BF16, kind="ExternalInput").ap()

    ctxo = nc.dram_tensor("ctxo", [NB, E], F32, kind="ExternalOutput").ap()
    awo = nc.dram_tensor("awo", [NB, T], F32, kind="ExternalOutput").ap()

    with tile.TileContext(nc) as tc:
        with (
            tc.tile_pool(name="sb", bufs=1) as sb,
            tc.tile_pool(name="ps_en", bufs=1, space="PSUM") as ps_en_pool,
        ):
            # ---------------- phase 0: constants -------------------------
            wenc_sb = sb.tile([128, KE, A], BF16, tag="wenc")
            nc.sync.dma_start(out=wenc_sb, in_=wenc.rearrange("(k p) a -> p k a", p=128))
            wdec_sb = sb.tile([128, KD, A], BF16, tag="wdec")
            nc.sync.dma_start(out=wdec_sb, in_=wdec.rearrange("(k p) a -> p k a", p=128))
            g1_sb = sb.tile([128, A], BF16, tag="g1")
            nc.sync.dma_start(out=g1_sb, in_=gmat[0:128, :])
            g2_sb = sb.tile([KW2, A], BF16, tag="g2")
            nc.sync.dma_start(out=g2_sb, in_=gmat[128:KW, :])
            v_sb = sb.tile([128, KA], BF16, tag="v")
            nc.sync.dma_start(out=v_sb, in_=vt)
            benc_sb = sb.tile([1, A], BF16, tag="benc")
            nc.sync.dma_start(out=benc_sb, in_=benc)
            dec32 = sb.tile([NB, D], F32, tag="dec32")
            nc.sync.dma_start(out=dec32, in_=dec)

            msk_sb = sb.tile([128, T], F32, tag="msk")
            nc.vector.memset(msk_sb, 0.0)
            for b in range(NB):
                nc.sync.dma_start(
                    out=msk_sb[32 * b : 32 * b + 1, :], in_=msk[b : b + 1, :]
                )

            ident = sb.tile([128, 128], BF16, tag="ident")
            make_identity(nc, ident)
            ones4 = sb.tile([1, NB], BF16, tag="ones4")
            nc.vector.memset(ones4, 1.0)

            # energy accumulators [128, 512] fp32, one bank per chunk;
            # memset so junk partitions read as 0 later.
            ps_en = [
                ps_en_pool.tile(
                    [128, 512], F32, tag=f"en{c}", bufs=1, name=f"ps_en{c}"
                )
                for c in range(NCH)
            ]
            for c in range(NCH):
                nc.vector.memset(ps_en[c], 0.0)

            # ---- dec bias: bias[a, b] = (dec @ W_dec)[b, a] + b_enc[a]
            bias_sb = sb.tile([128, KA, NB], F32, tag="bias")
            with tc.tile_pool(name="ps0", bufs=1, space="PSUM") as ps0:
                decbf = sb.tile([NB, D], BF16, tag="decbf")
                nc.vector.tensor_copy(decbf, dec32)
                ps_dec = ps0.tile([128, KD, NB], BF16, tag="psdec", bufs=1)
                for kd in range(KD):
                    nc.tensor.transpose(
                        ps_dec[:, kd, :],
                        decbf[0:NB, 128 * kd : 128 * (kd + 1)],
                        ident[0:NB, 0:NB],
                    )
                decT = sb.tile([128, KD, NB], BF16, tag="decT")
                nc.vector.tensor_copy(decT, ps_dec)

                for ka in range(KA):
                    ps_b = ps0.tile([128, NB], F32, tag="psbias", bufs=1)
                    for kd in range(KD):
                        nc.tensor.matmul(
                            ps_b,
                            wdec_sb[:, kd, 128 * ka : 128 * (ka + 1)],
                            decT[:, kd, :],
                            start=(kd == 0),
                            stop=False,
                        )
                    nc.tensor.matmul(
                        ps_b,
                        benc_sb[0:1, 128 * ka : 128 * (ka + 1)],
                        ones4[0:1, :],
                        start=False,
                        stop=True,
                    )
                    nc.vector.tensor_copy(bias_sb[:, ka, :], ps_b)

            with tc.tile_pool(name="ps1", bufs=1, space="PSUM") as ps1:
                # ---------------- phase A: per-batch compute -----------------
                encn = sb.tile([128, NB, NT, E], BF16, tag="encn")
                for b in range(NB):
                    # enc natural tiles (cast fp32 -> bf16 during DMA)
                    nc.gpsimd.dma_start(
                        out=encn[:, b, 0:11, :],
                        in_=enc[b, 0:1408, :].rearrange("(n p) e -> p n e", p=128),
                    )
                    nc.gpsimd.dma_start(
                        out=encn[0:92, b, 11, :], in_=enc[b, 1408:1500, :]
                    )
                    # Hankel tiles S[k, t] = aw_pad[t + k]
                    hank1 = sb.tile([128, T], BF16, tag="hank1", bufs=2)
                    nc.gpsimd.dma_start(
                        out=hank1, in_=AP(awp.tensor, b * TPAD, [[1, 128], [1, T]])
                    )
                    hank2 = sb.tile([KW2, T], BF16, tag="hank2", bufs=2)
                    nc.gpsimd.dma_start(
                        out=hank2,
                        in_=AP(awp.tensor, b * TPAD + 128, [[1, KW2], [1, T]]),
                    )

                    encT = sb.tile([128, KE, T], BF16, tag="encT", bufs=2)
                    preT = sb.tile([128, KA, T], BF16, tag="preT", bufs=2)
                    for c in range(NCH):
                        t0 = CH * c
                        for n in CHUNK_TILES[c]:
                            tsz = TSZ[n]
                            ps_tr = ps1.tile([128, KE, 128], BF16, tag="pstr", bufs=2)
                            for ke in range(KE):
                                nc.tensor.transpose(
                                    ps_tr[:, ke, 0:tsz],
                                    encn[0:tsz, b, n, 128 * ke : 128 * (ke + 1)],
                                    ident[0:tsz, 0:tsz],
                                )
                            nc.vector.tensor_copy(
                                encT[:, :, 128 * n : 128 * n + tsz],
                                ps_tr[:, :, 0:tsz],
                            )
                        for ka in range(KA):
                            ps_pre = ps1.tile([128, CH], F32, tag="pspre", bufs=2)
                            for ke in range(KE):
                                nc.tensor.matmul(
                                    ps_pre,
                                    wenc_sb[:, ke, 128 * ka : 128 * (ka + 1)],
                                    encT[:, ke, t0 : t0 + CH],
                                    start=(ke == 0),
                                    stop=False,
                                )
                            nc.tensor.matmul(
                                ps_pre,
                                g1_sb[:, 128 * ka : 128 * (ka + 1)],
                                hank1[:, t0 : t0 + CH],
                                start=False,
                                stop=False,
                            )
                            nc.tensor.matmul(
                                ps_pre,
                                g2_sb[:, 128 * ka : 128 * (ka + 1)],
                                hank2[:, t0 : t0 + CH],
                                start=False,
                                stop=True,
                            )
                            nc.scalar.activation(
                                preT[:, ka, t0 : t0 + CH],
                                ps_pre,
                                mybir.ActivationFunctionType.Tanh,
                                bias=bias_sb[:, ka, b : b + 1],
                            )
                        # energy row for this (batch, chunk)
                        for ka in range(KA):
                            nc.tensor.matmul(
                                ps_en[c][32 * b : 32 * b + 1, 0:CH],
                                v_sb[:, ka : ka + 1],
                                preT[:, ka, t0 : t0 + CH],
                                start=(ka == 0),
                                stop=(ka == KA - 1),
                                tile_position=(0, 32 * b),
                            )

            # ---------------- phase B: masked softmax (all batches) ------
            energy = sb.tile([128, T], F32, tag="energy")
            for c in range(NCH):
                nc.scalar.copy(energy[:, CH * c : CH * (c + 1)], ps_en[c][:, 0:CH])
            nc.vector.tensor_mul(energy, energy, msk_sb)
            negmx = sb.tile([128, 1], F32, tag="negmx")
            nc.vector.tensor_reduce(
                negmx, energy, axis=mybir.AxisListType.X,
                op=mybir.AluOpType.max, negate=True,
            )
            pexp = sb.tile([128, T], F32, tag="pexp")
            ssum = sb.tile([128, 1], F32, tag="ssum")
            nc.scalar.activation(
                pexp, energy, mybir.ActivationFunctionType.Exp,
                bias=negmx[:, 0:1], accum_out=ssum,
            )
            rinv = sb.tile([128, 1], F32, tag="rinv")
            nc.vector.reciprocal(rinv, ssum)
            awf = sb.tile([128, T], F32, tag="awf")
            nc.vector.tensor_scalar(
                awf, pexp, scalar1=rinv[:, 0:1], scalar2=None,
                op0=mybir.AluOpType.mult,
            )
            awbf = sb.tile([128, T], BF16, tag="awbf")
            nc.vector.tensor_copy(awbf, awf)
            for b in range(NB):
                nc.sync.dma_start(
                    out=awo[b : b + 1, :], in_=awf[32 * b : 32 * b + 1, :]
                )

            # ---------------- phase C: context ---------------------------
            with tc.tile_pool(name="ps2", bufs=1, space="PSUM") as ps2:
                awT = sb.tile([128, NT, NB], BF16, tag="awT")
                for n in range(NT):
                    tsz = TSZ[n]
                    ps_awt = ps2.tile([128, 128], BF16, tag="psawt", bufs=2)
                    nc.tensor.transpose(
                        ps_awt[0:tsz, :], awbf[:, 128 * n : 128 * n + tsz], ident
                    )
                    nc.vector.tensor_copy(awT[0:tsz, n, :], ps_awt[0:tsz, 0:128:32])
                ps_ctx = ps2.tile([128, E], F32, tag="psctx", bufs=1)
                nc.vector.memset(ps_ctx, 0.0)
                for b in range(NB):
                    for n in range(NT):
                        tsz = TSZ[n]
                        nc.tensor.matmul(
                            ps_ctx[32 * b : 32 * b + 1, :],
                            awT[0:tsz, n, b : b + 1],
                            encn[0:tsz, b, n, :],
                            start=(n == 0),
                            stop=(n == NT - 1),
                            tile_position=(0, 32 * b),
                        )
                ctx_sb = sb.tile([128, E], F32, tag="ctxsb")
                nc.scalar.copy(ctx_sb, ps_ctx)
                for b in range(NB):
                    nc.sync.dma_start(
                        out=ctxo[b : b + 1, :], in_=ctx_sb[32 * b : 32 * b + 1, :]
                    )

    nc.compile()
    return nc


_NC = None


def _get_nc():
    global _NC
    if _NC is None:
        _NC = _build()
    return _NC


def kernel(enc_out, dec_out, aw_step, W_enc, b_enc, W_dec, W_conv, V, conv_w, x_lens):
    global LAST_EXEC_NS, LAST_RESULTS
    enc_out = np.asarray(enc_out, dtype=np.float32)
    dec_out = np.asarray(dec_out, dtype=np.float32)
    aw_step = np.asarray(aw_step, dtype=np.float32)
    W_enc = np.asarray(W_enc, dtype=np.float32)
    b_enc = np.asarray(b_enc, dtype=np.float32)
    W_dec = np.asarray(W_dec, dtype=np.float32)
    W_conv = np.asarray(W_conv, dtype=np.float32)
    V = np.asarray(V, dtype=np.float32)
    conv_w = np.asarray(conv_w, dtype=np.float32)
    x_lens = np.asarray(x_lens, dtype=np.int32)

    # host-side weight folding / packing (weights only, no data compute)
    G = np.einsum(
        "ck,ca->ka",
        conv_w[:, 0, :].astype(np.float64),
        W_conv.astype(np.float64),
    ).astype(np.float32)
    awp = np.zeros((B, TPAD), dtype=np.float32)
    awp[:, 100 : 100 + T] = aw_step[:, :, 0]
    mask = (np.arange(T)[None, :] < x_lens[:, None]).astype(np.float32)
    vt = np.ascontiguousarray(V[:, 0].reshape(KA, 128).T)

    bf = ml_dtypes.bfloat16
    wenc_bf = W_enc.astype(bf)
    wdec_bf = W_dec.astype(bf)
    g_bf = G.astype(bf)
    vt_bf = vt.astype(bf)
    benc_bf = b_enc.reshape(1, A).astype(bf)
    awp_bf = awp.astype(bf)

    in_maps = []
    for i in range(NCORES):
        s = slice(NB * i, NB * (i + 1))
        in_maps.append(
            {
                "enc": np.ascontiguousarray(enc_out[s]),
                "dec": np.ascontiguousarray(dec_out[s, 0, :]),
                "awp": np.ascontiguousarray(awp_bf[s]),
                "msk": np.ascontiguousarray(mask[s]),
                "wenc": wenc_bf,
                "wdec": wdec_bf,
                "gmat": g_bf,
                "vt": vt_bf,
                "benc": benc_bf,
            }
        )

    nc = _get_nc()
    res = run_bass_kernel_spmd(
        nc,
        in_maps,
        list(range(NCORES)),
        trace=TRACE,
        trace_cores=[0] if TRACE else None,
    )
    LAST_EXEC_NS = res.exec_time_ns
    LAST_RESULTS = res

    context = np.concatenate([r["ctxo"] for r in res.results], axis=0)
    aw = np.concatenate([r["awo"] for r in res.results], axis=0)
    return context.reshape(B, 1, E), aw.reshape(B, T, 1)
